# revision 16
# baseline (speedup 1.0000x reference)
"""DC_CE_Marginal_loss for Trainium2 — 8-core data-parallel Bass kernel.

Shards the [B,C,D,H,W] volume along D across 8 NeuronCores, two launches:

  Launch A: each core loads its target shard (bf16; one-hot is exact) and
      computes local per-(b,c) voxel counts (free-dim reductions split over
      ScalarE and VectorE). Host sums the 8x[128,16] partials into global
      counts — the "psum of present-class counts" — and derives the
      presence masks / merge weights / CE padding (40 floats).

  Launch B: each core streams its net_output shard and computes per chunk:
      merged background logit (masked scalar_tensor_tensor chain), masked
      exp (ACT, additive -1e9 bias), softmax denominator S (pairwise adds),
      fast reciprocal, then fused affine_mul_reduce ops that produce
      softmax q while accumulating seg_vol / intersect / sum(t*m) into
      per-chunk columns; ACT Log accumulates sum(log(S+pad)).

Host sums the per-core/per-chunk partial columns and finishes the loss.
"""
import numpy as np
import ml_dtypes

B, C, D, H, W = 2, 8, 64, 160, 160
NCORES = 8
DS = D // NCORES            # depth slices per core
PLANE = DS * H * W          # voxels per (b,c) plane per core = 204800
P = 128
FREE = PLANE // P           # 1600
NCH = 4                     # chunks per sample plane
FCH = FREE // NCH           # 400
BIG = 1e9
NVOX = B * D * H * W

# launch B per-chunk accumulator columns: base = (b*NCH+ch)*CPC
CPC = 25          # seg[0:8], intersect[8:16], u-terms[16:24], lse[24]
NOUT = B * NCH * CPC
# masks input columns
MK_BM = 0         # 16: additive exp mask (0 present / -BIG absent)
MK_A = 16         # 16: 1-present (merge weights)
MK_PAD = 32       # 2: CE padding per sample
NMASK = 40

_CACHE = {}


def _build_a():
    import concourse.bacc as bacc
    import concourse.tile as tile
    from concourse import mybir

    FA = mybir.ActivationFunctionType
    AL = mybir.AluOpType
    f32, bf16 = mybir.dt.float32, mybir.dt.bfloat16

    nc = bacc.Bacc("TRN2", num_devices=NCORES, name="loss_counts")
    t = nc.dram_tensor("t", [B * C, P, FREE], bf16, kind="ExternalInput")
    out = nc.dram_tensor("cnt", [P, B * C], f32, kind="ExternalOutput")

    with tile.TileContext(nc) as tc:
        with (
            tc.tile_pool(name="tin", bufs=4) as tin,
            tc.tile_pool(name="sb", bufs=1) as sb,
        ):
            cnt = sb.tile([P, B * C], f32)
            junk_a = sb.tile([P, 2 * FREE], bf16)
            for g in range(8):  # 2 planes per DMA; reduce on DVE or ACT
                t_sb = tin.tile([P, 2, FREE], bf16, tag="t")
                src = t[2 * g : 2 * g + 2, :, :].rearrange("c p f -> p c f")
                nc.sync.dma_start(t_sb[:], src)
                if g % 2 == 0:
                    nc.vector.tensor_reduce(
                        out=cnt[:, 2 * g : 2 * g + 2], in_=t_sb[:],
                        axis=mybir.AxisListType.X, op=AL.add)
                else:
                    for j in range(2):
                        nc.scalar.activation(
                            out=junk_a[:, j * FREE : (j + 1) * FREE],
                            in_=t_sb[:, j, :], func=FA.Copy,
                            accum_out=cnt[:, 2 * g + j : 2 * g + j + 1])
            nc.sync.dma_start(out[:], cnt[:])
    nc.compile()
    return nc


import os
BG_GPSIMD = os.environ.get("K_BG_GPSIMD", "0") == "1"
E_BF16 = os.environ.get("K_E_BF16", "1") == "1"
I_ON_ACT = os.environ.get("K_I_ON_ACT", "1") == "1"
UM_GPSIMD = os.environ.get("K_UM_GPSIMD", "0") == "1"
UM_WIDE_TT = os.environ.get("K_UM_WIDE_TT", "1") == "1"


def _build_b():
    import concourse.bacc as bacc
    import concourse.tile as tile
    from concourse import mybir

    FA = mybir.ActivationFunctionType
    AL = mybir.AluOpType
    f32, bf16 = mybir.dt.float32, mybir.dt.bfloat16
    edt = bf16 if E_BF16 else f32

    nc = bacc.Bacc("TRN2", num_devices=NCORES, name="loss_main")
    x = nc.dram_tensor("x", [B * C, P, FREE], f32, kind="ExternalInput")
    t = nc.dram_tensor("t", [B * C, P, FREE], bf16, kind="ExternalInput")
    masks = nc.dram_tensor("masks", [P, NMASK], f32, kind="ExternalInput")
    out = nc.dram_tensor("out", [P, NOUT], f32, kind="ExternalOutput")

    beng = nc.gpsimd if BG_GPSIMD else nc.vector

    with tile.TileContext(nc) as tc:
        with (
            tc.tile_pool(name="persist", bufs=1) as persist,
            tc.tile_pool(name="xin", bufs=3) as xin,
            tc.tile_pool(name="ework", bufs=2) as ework,
            tc.tile_pool(name="qwork", bufs=2) as qwork,
            tc.tile_pool(name="swork", bufs=2) as swork,
        ):
            mk = persist.tile([P, NMASK], f32)
            nc.sync.dma_start(mk[:], masks[:])
            # prefetch chunk-0 logits before the (large) target loads so the
            # first chunk's DVE work isn't gated on all 6.6MB of t
            x_ch0 = xin.tile([P, C, FCH], f32, tag="x", name="x_ch0")
            nc.sync.dma_start(
                x_ch0[:], x[0:C, :, 0:FCH].rearrange("c p f -> p c f"))
            t_sb = persist.tile([P, B * C, FREE], bf16)
            for bc in range(B * C):
                nc.sync.dma_start(t_sb[:, bc, :], t[bc])
            accs = persist.tile([P, NOUT], f32)
            nc.vector.memset(accs[:], 0.0)
            junk_dve = persist.tile([P, C, FCH], f32)
            # all S chunks retained so the Ln ops run back-to-back at the
            # end (one act-table load instead of per-chunk exp<->ln flips)
            S_all = persist.tile([P, B * NCH, FCH], f32)

            for b in range(B):
                for ch in range(NCH):
                    sl = slice(ch * FCH, (ch + 1) * FCH)
                    base = (b * NCH + ch) * CPC
                    if b == 0 and ch == 0:
                        x_ch = x_ch0
                    else:
                        x_ch = xin.tile([P, C, FCH], f32, tag="x")
                        src = x[b * C : (b + 1) * C, :, sl].rearrange(
                            "c p f -> p c f")
                        nc.sync.dma_start(x_ch[:], src)

                    # bg = sum_{c>=1} absent_c * x_c ; x_0 += bg (merged logit)
                    bg = swork.tile([P, FCH], f32, tag="bg")
                    beng.tensor_scalar(
                        bg[:], x_ch[:, 1, :],
                        mk[:, MK_A + b * C + 1 : MK_A + b * C + 2], None, AL.mult)
                    for c in range(2, C):
                        bg2 = swork.tile([P, FCH], f32, tag="bg")
                        beng.scalar_tensor_tensor(
                            out=bg2[:], in0=x_ch[:, c, :],
                            scalar=mk[:, MK_A + b * C + c : MK_A + b * C + c + 1],
                            in1=bg[:], op0=AL.mult, op1=AL.add)
                        bg = bg2
                    beng.scalar_tensor_tensor(
                        out=x_ch[:, 0, :], in0=x_ch[:, 0, :], scalar=1.0,
                        in1=bg[:], op0=AL.mult, op1=AL.add)

                    # e_c = exp(m_c + mask_bias_c)
                    e_ch = ework.tile([P, C, FCH], edt, tag="e")
                    for c in range(C):
                        last_exp = nc.scalar.activation(
                            out=e_ch[:, c, :], in_=x_ch[:, c, :],
                            func=FA.Exp,
                            bias=mk[:, MK_BM + b * C + c : MK_BM + b * C + c + 1],
                            scale=1.0)

                    # S = sum_c e_c (pairwise tree on wide slices)
                    s4 = swork.tile([P, 4, FCH], edt, tag="s4")
                    nc.vector.tensor_tensor(out=s4[:], in0=e_ch[:, 0:4, :],
                                            in1=e_ch[:, 4:8, :], op=AL.add)
                    s2 = swork.tile([P, 2, FCH], edt, tag="s2")
                    nc.vector.tensor_tensor(out=s2[:], in0=s4[:, 0:2, :],
                                            in1=s4[:, 2:4, :], op=AL.add)
                    S = S_all[:, b * NCH + ch, :]
                    nc.vector.tensor_tensor(out=S, in0=s2[:, 0, :],
                                            in1=s2[:, 1, :], op=AL.add)

                    r = swork.tile([P, FCH], f32, tag="r")
                    nc.vector.reciprocal_approx_fast(r[:], S)

                    # q_c = e_c * r ; seg_c = sum(q_c)  (fused custom DVE op)
                    q_ch = qwork.tile([P, C, FCH], edt, tag="q")
                    for c in range(C):
                        nc.vector.affine_mul_reduce(
                            out=q_ch[:, c, :],
                            accum_out=accs[:, base + c : base + c + 1],
                            in0=e_ch[:, c, :], in1=r[:], scale=1.0, bias=0.0)
                    # intersect_c = sum(t_c * q_c)
                    if I_ON_ACT:
                        tq_ch = qwork.tile([P, C, FCH], edt, tag="tq")
                        nc.vector.tensor_tensor(
                            out=tq_ch[:], in0=t_sb[:, b * C : (b + 1) * C, sl],
                            in1=q_ch[:], op=AL.mult)
                        for c in range(C):
                            nc.scalar.activation(
                                out=tq_ch[:, c, :], in_=tq_ch[:, c, :],
                                func=FA.Copy,
                                accum_out=accs[:, base + 8 + c : base + 9 + c])
                    else:
                        for c in range(C):
                            nc.vector.affine_mul_reduce(
                                out=junk_dve[:, 0, :],
                                accum_out=accs[:, base + 8 + c : base + 9 + c],
                                in0=t_sb[:, b * C + c, sl], in1=q_ch[:, c, :],
                                scale=1.0, bias=0.0)
                    # u-term = sum_c sum(t_c * m_c)   (x_0 already merged)
                    if UM_WIDE_TT:
                        um_ch = qwork.tile([P, C, FCH], f32, tag="um")
                        ueng = nc.gpsimd if UM_GPSIMD else nc.vector
                        ueng.tensor_tensor(
                            out=um_ch[:], in0=t_sb[:, b * C : (b + 1) * C, sl],
                            in1=x_ch[:, :, :], op=AL.mult)
                        nc.scalar.activation(
                            out=um_ch[:], in_=um_ch[:], func=FA.Copy,
                            accum_out=accs[:, base + 16 : base + 17])
                    else:
                        for c in range(C):
                            nc.vector.affine_mul_reduce(
                                out=junk_dve[:, 0, :],
                                accum_out=accs[:, base + 16 + c : base + 17 + c],
                                in0=t_sb[:, b * C + c, sl],
                                in1=x_ch[:, c, :],
                                scale=1.0, bias=0.0)

            # CE lse terms at the end: sum(log(S + pad_b)) via ACT accum
            junk_act = persist.tile([P, FCH], f32)
            from concourse.tile import add_dep_helper
            for b in range(B):
                for ch in range(NCH):
                    base = (b * NCH + ch) * CPC
                    ln_inst = nc.scalar.activation(
                        out=junk_act[:], in_=S_all[:, b * NCH + ch, :],
                        func=FA.Ln,
                        bias=mk[:, MK_PAD + b : MK_PAD + b + 1], scale=1.0,
                        accum_out=accs[:, base + 24 : base + 25])
                    # keep every Ln after the final Exp so the activation
                    # table set is switched exactly once
                    add_dep_helper(ln_inst.ins, last_exp.ins, False,
                                   "batch ln after exps")

            nc.sync.dma_start(out[:], accs[:])
    nc.compile()
    return nc


def _get(name, builder):
    if name not in _CACHE:
        _CACHE[name] = builder()
    return _CACHE[name]


def _shard_inputs(net_output, target):
    xs = np.ascontiguousarray(net_output).reshape(B, C, NCORES, P, FREE)
    ts = np.ascontiguousarray(target).reshape(B, C, NCORES, P, FREE)
    xmaps, tmaps = [], []
    for k in range(NCORES):
        xk = np.ascontiguousarray(xs[:, :, k]).reshape(B * C, P, FREE)
        tk = np.ascontiguousarray(ts[:, :, k]).reshape(B * C, P, FREE)
        xmaps.append(xk)
        tmaps.append(tk.astype(ml_dtypes.bfloat16))  # one-hot: exact in bf16
    return xmaps, tmaps


def _masks_from_counts(cnt_g):
    """cnt_g [B,C] -> (masks [P,NMASK] f32, present, n)"""
    present = cnt_g > 0.5
    pm = present.astype(np.float32)
    n = pm.sum(axis=1)
    L = n.max()
    pad = (L - n).astype(np.float32)
    mrow = np.zeros((NMASK,), dtype=np.float32)
    mrow[MK_BM : MK_BM + B * C] = pm.reshape(-1) * BIG - BIG
    mrow[MK_A : MK_A + B * C] = 1.0 - pm.reshape(-1)
    mrow[MK_PAD : MK_PAD + B] = pad
    masks = np.ascontiguousarray(np.broadcast_to(mrow, (P, NMASK)))
    return masks, present, n


def _run(nc, in_maps, out_name):
    if os.environ.get("K_SIM", "0") == "1":
        import concourse.bass_interp as bass_interp
        sim = bass_interp.MultiCoreSim(nc, NCORES)
        for k in range(NCORES):
            for name, arr in in_maps[k].items():
                sim.cores[k].tensor(name)[:] = arr
        sim.simulate()
        return [{out_name: sim.cores[k].tensor(out_name).copy()}
                for k in range(NCORES)]
    from concourse.bass_utils import run_bass_kernel_spmd
    return run_bass_kernel_spmd(
        nc, in_maps, core_ids=list(range(NCORES))).results


def run_a(tmaps):
    nc = _get("a", _build_a)
    results = _run(nc, [{"t": tk} for tk in tmaps], "cnt")
    cnt_g = np.zeros((B, C), dtype=np.float64)
    for r in results:
        cnt_g += r["cnt"].astype(np.float64).sum(axis=0).reshape(B, C)
    return cnt_g


def run_b(xmaps, tmaps, masks):
    nc = _get("b", _build_b)
    in_maps = [{"x": xmaps[k], "t": tmaps[k], "masks": masks}
               for k in range(NCORES)]
    results = _run(nc, in_maps, "out")
    acc = np.zeros((NOUT,), dtype=np.float64)
    for r in results:
        acc += r["out"].astype(np.float64).sum(axis=0)
    return acc


def _finish(cnt_g, acc, present, n):
    cols = acc.reshape(B, NCH, CPC).sum(axis=1)   # [B, CPC]
    seg = cols[:, 0:8]
    inter = cols[:, 8:16]
    u = cols[:, 16:24].sum(axis=1)                # [B]
    lse_sum = cols[:, 24]
    ce = (lse_sum.sum() - u.sum()) / NVOX
    dice_c = 2.0 * inter / (cnt_g + seg + 1e-5)
    dice_i = 1.0 - (present * dice_c).sum(axis=1) / n
    dc = dice_i.mean()
    return np.asarray(0.5 * ce + 0.5 * dc, dtype=np.float32)


def kernel(net_output, target):
    xmaps, tmaps = _shard_inputs(np.asarray(net_output), np.asarray(target))
    cnt_g = run_a(tmaps)
    masks, present, n = _masks_from_counts(cnt_g)
    acc = run_b(xmaps, tmaps, masks)
    return _finish(cnt_g, acc, present, n)
